# revision 47
# baseline (speedup 1.0000x reference)
"""FCOS head (nn_FCOS_73787538145418) Trainium2 Bass kernel.

Sharding: data-parallel, one image per NeuronCore (B=8 across 8 cores),
weights replicated. Each core runs the identical SPMD NEFF over its image.

Stem convs (two 4-layer 3x3 conv 256->256 + ReLU stems per level) run as
1D Winograd F(2,3) along x: per output-column pair, 4 Winograd points
m0..m3 accumulate in separate PSUM banks over (2 ci chunks x 3 ky taps)
K=128 matmuls on bf16 transformed inputs/weights (1.5x fewer PE rows than
direct conv), then y0=relu(m0+m1+m2+b), y1=relu(m1-m2-m3+b) recombine on
the vector/gpsimd/scalar engines while the PE streams the next band.
Towers are stored bf16 (halves transform read cost + DVE write cost).
Prediction convs (cls 20ch; box+ctr 5ch) run direct bf16, 4-way packed
on the PE via tile_position col-groups: each tile's rows split in half;
cls halves accumulate concurrently in col-strips 0/2 (PSUM partitions
0:20 / 64:84) and box+ctr halves in strips 1/3 (32:37 / 96:101). The
four strips stream concurrently (~4 cols/cycle), the per-strip halves
cover different output pixels so no cross-partition combine is needed
(~4x pred throughput vs serial taps). Both stems' layer 0 share one
input transform (same source features).

Hard-won scheduling facts: tensor_tensor may read at most ONE PSUM
operand; gpsimd cannot touch PSUM at all and runs TT at ~half DVE rate;
engines are lane-aligned (no cross-partition compute), so each packed
pred stream keeps all its taps in ONE col group; DMA writes to one tile
are WAW-ordered across queues (ship them on one queue in need-order);
weight pool tiles must be DMA'd on the sync queue; per-matmul issue
overhead is ~6-26ns, LDWEIGHTS (FWL) hides under the matmul stream.
"""
import sys

if '/opt/trn_rl_repo' not in sys.path:
    sys.path.insert(0, '/opt/trn_rl_repo')

import numpy as np
import ml_dtypes

import concourse.mybir as mybir
from concourse import bacc
import concourse.tile as tile
from concourse.bass_utils import run_bass_kernel_spmd

P = 128
NCH = 2                 # 256 channels = 2 chunks of 128
C = 256
NL = 4                  # stem depth
NPIX_TOTAL = 5376
F32 = mybir.dt.float32
BF16 = mybir.dt.bfloat16
ADD = mybir.AluOpType.add
SUB = mybir.AluOpType.subtract
RELU = mybir.ActivationFunctionType.Relu

_cached = {}
_run_opts = {}   # extra kwargs for run_bass_kernel_spmd (test harness: trace)
_last = {}       # last BassKernelResults (test harness reads exec_time_ns)


def _pad_view(flat_tile, off, H, W):
    n = NCH * (H + 2) * (W + 2)
    return flat_tile[:, off:off + n].rearrange(
        "p (c h w) -> p c h w", c=NCH, h=H + 2, w=W + 2)


def _pair_view(flat_tile, off, H, W):
    n = NCH * (H + 2) * (W + 2)
    return flat_tile[:, off:off + n].rearrange(
        "p (c h x two) -> p c h x two",
        c=NCH, h=H + 2, x=(W + 2) // 2, two=2)


def _zero_ring(nc, v, H, W):
    for c in range(NCH):
        nc.vector.memset(v[:, c, 0, :], 0.0)
        nc.vector.memset(v[:, c, H + 1, :], 0.0)
        nc.vector.memset(v[:, c, 1:H + 1, 0], 0.0)
        nc.vector.memset(v[:, c, 1:H + 1, W + 1], 0.0)


class _WL:
    """One Winograd stem conv layer (3x3 same, 256->256, + bias + ReLU)."""

    def __init__(self, nc, wpool, upool, spool, psum, vw_d, sbias,
                 s, l, src_pv, dst_pv, H, W, tag, fine_tf=False, rb=None):
        self.nc = nc
        self.wpool, self.upool, self.spool, self.psum = \
            wpool, upool, spool, psum
        self.vw_d, self.sbias = vw_d, sbias
        self.s, self.l = s, l
        self.src, self.dst = src_pv, dst_pv
        self.H, self.W, self.tag = H, W, tag
        self.fine_tf = fine_tf
        self.RB = rb or (16 if H == 64 else H)   # band rows
        self.NB = H // self.RB              # bands
        self.TX = W // 2                    # tiles per row
        self.RR = self.RB + 2               # U rows per band
        self.wts = {}
        self.us = {}
        self.ms = {}

    def weights_pt(self, pt, eng=None):
        if pt in self.wts:
            return
        nc = self.nc
        eng = eng or nc.sync
        wt = self.wpool.tile([P, NCH, NCH, 3, P], BF16, tag="ww",
                             name=f"ww_{self.tag}_{pt}")
        eng.dma_start(wt[:], self.vw_d[self.s, self.l, pt])
        self.wts[pt] = wt

    def weights(self, eng=None):
        for pt in range(4):
            self.weights_pt(pt, eng)

    def tf(self, q):
        """Input transform for band q: U[c, r, pt, tx] (bf16).

        One op per Winograd point covering both ci chunks; pt0-2 on the
        vector engine, pt3 on gpsimd (gpsimd TT runs at ~half DVE rate
        and also carries the tP/tM output-transform combines)."""
        if q in self.us:
            return
        nc = self.nc
        TX = self.TX
        u = self.upool.tile([P, NCH, self.RR, 4, TX], BF16, tag="u",
                            name=f"u_{self.tag}_{q}")
        r0 = self.RB * q
        rows = slice(r0, r0 + self.RR)
        if self.fine_tf:
            # startup-critical: smaller ops, spread over both engines
            for c in range(NCH):
                e0 = self.src[:, c, rows, 0:TX, 0]
                e1 = self.src[:, c, rows, 1:TX + 1, 0]
                o0 = self.src[:, c, rows, 0:TX, 1]
                o1 = self.src[:, c, rows, 1:TX + 1, 1]
                nc.vector.tensor_tensor(u[:, c, :, 0], e0, e1, SUB)
                nc.vector.tensor_tensor(u[:, c, :, 1], o0, e1, ADD)
                nc.vector.tensor_tensor(u[:, c, :, 2], e1, o0, SUB)
                nc.gpsimd.tensor_tensor(u[:, c, :, 3], o0, o1, SUB)
        else:
            e0 = self.src[:, :, rows, 0:TX, 0]
            e1 = self.src[:, :, rows, 1:TX + 1, 0]
            o0 = self.src[:, :, rows, 0:TX, 1]
            o1 = self.src[:, :, rows, 1:TX + 1, 1]
            nc.vector.tensor_tensor(u[:, :, :, 0], e0, e1, SUB)
            nc.gpsimd.tensor_tensor(u[:, :, :, 1], o0, e1, ADD)
            nc.gpsimd.tensor_tensor(u[:, :, :, 2], e1, o0, SUB)
            nc.gpsimd.tensor_tensor(u[:, :, :, 3], o0, o1, SUB)
        self.us[q] = u

    def mm(self, b):
        nc = self.nc
        u = self.us[b]
        self.ms[b] = {}
        sz = self.RB * self.TX
        for o in range(NCH):
            if sz <= 256:
                # half-band mode: pack two winograd points per 2KB bank
                # (pool PSUM allocation is bank-granular) so a paired
                # layer's matmuls can overlap in the other 4 banks.
                # Pair (m1,m2) / (m0,m3): freed together by the out-tf.
                slot = {1: 0, 2: 1, 0: 2, 3: 3}
                banks = [self.psum.tile([P, 512], F32, tag="ps",
                                        name=f"mb_{self.tag}_{b}_{o}_{i}")
                         for i in range(2)]
                self.ms[b][o] = [
                    banks[slot[pt] // 2]
                    [:, (slot[pt] % 2) * sz:(slot[pt] % 2) * sz + sz]
                    .rearrange("p (r x) -> p r x", r=self.RB, x=self.TX)
                    for pt in range(4)]
            else:
                self.ms[b][o] = [
                    self.psum.tile([P, self.RB, self.TX], F32, tag="ps",
                                   name=f"m_{self.tag}_{b}_{o}_{pt}")
                    for pt in range(4)]
        # pt-outer order: each point's weight tile is only needed 2*NCH
        # matmuls later than with o-outer, giving the weight-DMA prefetch
        # ~2.6us more slack (kills the startup/layer-boundary PE gaps)
        for pt in range(4):
            wt = self.wts[pt]
            for o in range(NCH):
                k = 0
                for c in range(NCH):
                    for ky in range(3):
                        nc.tensor.matmul(self.ms[b][o][pt][:],
                                         wt[:, c, o, ky],
                                         u[:, c, ky:ky + self.RB, pt],
                                         start=(k == 0), stop=(k == 5))
                        k += 1

    def outtf(self, b):
        """y0 = relu(m0+m1+m2+b) -> odd cols; y1 = relu(m1-m2-m3+b) -> even."""
        nc = self.nc
        RB, TX = self.RB, self.TX
        rows = slice(1 + RB * b, 1 + RB * (b + 1))
        for o in range(NCH):
            m0, m1, m2, m3 = self.ms[b][o]
            t = f"{self.tag}_{b}_{o}"
            c2 = self.spool.tile([P, RB, TX], F32, tag="scf", name=f"c2_{t}")
            tP = self.spool.tile([P, RB, TX], F32, tag="scf", name=f"tp_{t}")
            tM = self.spool.tile([P, RB, TX], F32, tag="scf", name=f"tm_{t}")
            r0 = self.spool.tile([P, RB, TX], F32, tag="scf", name=f"r0_{t}")
            r1 = self.spool.tile([P, RB, TX], F32, tag="scf", name=f"r1_{t}")
            bias = self.sbias[:, self.s, self.l, o]
            # short PSUM-freeing chain on ONE fast engine: a long
            # cross-engine chain here stalls the next layer's matmuls on
            # PSUM banks and lets HAM re-throttle the PE (measured -120us)
            nc.scalar.copy(c2[:], m2[:])
            nc.vector.tensor_tensor(tP[:], m1[:], c2[:], ADD)
            nc.vector.tensor_tensor(tM[:], m1[:], c2[:], SUB)
            nc.vector.tensor_tensor(r0[:], m0[:], tP[:], ADD)
            nc.vector.tensor_tensor(r1[:], m3[:], tM[:], SUB)   # m3 - tM
            nc.scalar.activation(self.dst[:, o, rows, 0:TX, 1], r0[:],
                                 RELU, bias=bias)
            nc.scalar.activation(self.dst[:, o, rows, 1:TX + 1, 0], r1[:],
                                 RELU, bias=bias, scale=-1.0)
        del self.ms[b]


def _emit_chain(layers, post_hooks=None):
    """Emit a list of _WL layers sequentially with next-layer tf hoisting."""
    post_hooks = post_hooks or {}
    n = len(layers)
    for i, L in enumerate(layers):
        nxt = layers[i + 1] if i + 1 < n else None
        L.weights()
        L.tf(0)
        if L.NB > 1:
            L.tf(1)
        for b in range(L.NB):
            L.mm(b)
            if b + 2 < L.NB:
                L.tf(b + 2)
            if b == L.NB - 1 and nxt is not None:
                nxt.weights()
                nxt.tf(0)
            L.outtf(b)
            if b == L.NB - 1 and nxt is not None and nxt.NB > 1:
                nxt.tf(1)
        if i in post_hooks:
            post_hooks[i]()


def _emit_pair(Lc, Lb, nxt=None):
    """Emit two stem layers that read the SAME source (both stems'
    layer 0), sharing one input transform: Lb reuses Lc's U tiles and
    the two layers' matmuls/out-transforms interleave band-by-band."""
    Lb.us = Lc.us     # share the U dict by reference
    Lc.weights()
    Lb.weights()
    Lc.tf(0)
    if Lc.NB > 1:
        Lc.tf(1)
    for b in range(Lc.NB):
        Lc.mm(b)
        if b + 2 < Lc.NB:
            Lc.tf(b + 2)
        Lc.outtf(b)
        Lb.mm(b)
        if b == Lc.NB - 1 and nxt is not None:
            nxt.weights()
        Lb.outtf(b)
    if nxt is not None:
        nxt.tf(0)
        if nxt.NB > 1:
            nxt.tf(1)


def _preds4(nc, psum_pool, stage_pool, pwc, pwb, pbc, pbb,
            tcls, tbox, out_d, H, W, R, pix_base, tag):
    """Packed prediction convs, 4 concurrent PE col-strips per tile:
    each tile's rows split in half; cls halves accumulate in strips 0/2
    (PSUM partitions 0:20 / 64:84), box+ctr halves in strips 1/3
    (32:37 / 96:101). Different output pixels per strip, so no
    cross-partition combine is needed — ACT adds bias per strip and the
    DMAs reassemble rows in DRAM."""
    R2 = R // 2
    n_tiles = H // R
    IDENT = mybir.ActivationFunctionType.Identity
    for it in range(n_tiles):
        rr = it * R
        ps = psum_pool.tile([P, R, W], F32, tag="ps", name=f"pf_{tag}_{it}")
        for k in range(18):
            c, t = k // 9, k % 9
            ky, kx = t // 3, t % 3
            rc0 = tcls[:, c, rr + ky:rr + ky + R2, kx:kx + W]
            rc1 = tcls[:, c, rr + R2 + ky:rr + R2 + ky + R2, kx:kx + W]
            rb0 = tbox[:, c, rr + ky:rr + ky + R2, kx:kx + W]
            rb1 = tbox[:, c, rr + R2 + ky:rr + R2 + ky + R2, kx:kx + W]
            st_, sp_ = (k == 0), (k == 17)
            nc.tensor.matmul(ps[0:20, 0:R2], pwc[:, c, t], rc0,
                             start=st_, stop=sp_, tile_position=(0, 0))
            nc.tensor.matmul(ps[32:37, 0:R2], pwb[:, c, t], rb0,
                             start=st_, stop=sp_, tile_position=(0, 32))
            nc.tensor.matmul(ps[64:84, R2:R], pwc[:, c, t], rc1,
                             start=st_, stop=sp_, tile_position=(0, 64))
            nc.tensor.matmul(ps[96:101, R2:R], pwb[:, c, t], rb1,
                             start=st_, stop=sp_, tile_position=(0, 96))
        st = stage_pool.tile([P, R * W], F32, tag="st", name=f"st_{tag}_{it}")
        n2 = R2 * W
        c0 = pix_base + rr * W
        for half, (pc, pb) in enumerate([(0, 32), (64, 96)]):
            vc = ps[pc:pc + 20, half * R2:half * R2 + R2].rearrange(
                "p r w -> p (r w)")
            vb = ps[pb:pb + 5, half * R2:half * R2 + R2].rearrange(
                "p r w -> p (r w)")
            sc = st[pc:pc + 20, half * n2:half * n2 + n2]
            sb = st[pb:pb + 5, half * n2:half * n2 + n2]
            nc.scalar.activation(sc, vc, IDENT, bias=pbc[pc:pc + 20])
            nc.scalar.activation(sb, vb, IDENT, bias=pbb[pb:pb + 5])
            nc.sync.dma_start(
                out_d[0:20, c0 + half * n2:c0 + half * n2 + n2], sc)
            nc.sync.dma_start(
                out_d[20:25, c0 + half * n2:c0 + half * n2 + n2], sb)


def _build():
    nc = bacc.Bacc("TRN2", target_bir_lowering=False, debug=False,
                   num_devices=8)

    # all features ship bf16 (they only feed winograd transforms / preds)
    x_d = [nc.dram_tensor("x0", (P, NCH, 66, 66), BF16,
                          kind="ExternalInput"),
           nc.dram_tensor("x1", (P, NCH, 34, 34), BF16,
                          kind="ExternalInput"),
           nc.dram_tensor("x2", (P, NCH, 18, 18), BF16,
                          kind="ExternalInput")]
    vw_d = nc.dram_tensor("vw", (2, NL, 4, P, NCH, NCH, 3, P), BF16,
                          kind="ExternalInput")
    sb_d = nc.dram_tensor("sb", (2, NL, NCH, P, 1), F32, kind="ExternalInput")
    pwc_d = nc.dram_tensor("pwc", (P, NCH, 9, 20), BF16, kind="ExternalInput")
    pwb_d = nc.dram_tensor("pwb", (P, NCH, 9, 5), BF16, kind="ExternalInput")
    pbc_d = nc.dram_tensor("pbc", (20, 1), F32, kind="ExternalInput")
    pbb_d = nc.dram_tensor("pbb", (5, 1), F32, kind="ExternalInput")
    out_d = nc.dram_tensor("out", (25, NPIX_TOTAL), F32, kind="ExternalOutput")

    N3 = NCH * 66 * 66            # 8712: p3 padded elems/partition
    N4 = NCH * 34 * 34            # 2312
    N5 = NCH * 18 * 18            # 648

    with tile.TileContext(nc) as tc:
        with (
            tc.tile_pool(name="resident", bufs=1) as res_pool,
            tc.tile_pool(name="wwts", bufs=12) as wwts_pool,
            tc.tile_pool(name="upool", bufs=3) as upool,
            tc.tile_pool(name="scratch", bufs=8) as spool,
            tc.tile_pool(name="psum", bufs=8, space="PSUM") as psum_pool,
            tc.tile_pool(name="stage", bufs=4) as stage_pool,
        ):
            # p3 rotation buffers (bf16): A3 holds the cls tower, B3/C3
            # rotate for the box chain; feat3 holds the p3 features.
            padA3 = res_pool.tile([P, N3], BF16, name="padA3")
            padB3 = res_pool.tile([P, N3], BF16, name="padB3")
            padC3 = res_pool.tile([P, N3], BF16, name="padC3")
            feat3 = res_pool.tile([P, NCH, 66, 33, 2], BF16, name="feat3")
            # p4/p5 get their own (non-aliased) buffers so the scheduler
            # can overlap p4/p5 stems with the p3 preds.
            padA4 = res_pool.tile([P, N4], BF16, name="padA4")
            padB4 = res_pool.tile([P, N4], BF16, name="padB4")
            padC4 = res_pool.tile([P, N4], BF16, name="padC4")
            padA5 = res_pool.tile([P, N5], BF16, name="padA5")
            padB5 = res_pool.tile([P, N5], BF16, name="padB5")
            padC5 = res_pool.tile([P, N5], BF16, name="padC5")

            sbias = res_pool.tile([P, 2, NL, NCH, 1], F32, name="sbias")
            pwc = res_pool.tile([P, NCH, 9, 20], BF16, name="pwc")
            pwb = res_pool.tile([P, NCH, 9, 5], BF16, name="pwb")
            pbc = res_pool.tile([96, 1], F32, name="pbc")
            pbb = res_pool.tile([P, 1], F32, name="pbb")

            A3r, A3 = _pad_view(padA3, 0, 64, 64), _pair_view(padA3, 0, 64, 64)
            B3r, B3 = _pad_view(padB3, 0, 64, 64), _pair_view(padB3, 0, 64, 64)
            C3 = _pair_view(padC3, 0, 64, 64)
            A4r, A4 = _pad_view(padA4, 0, 32, 32), _pair_view(padA4, 0, 32, 32)
            B4r, B4 = _pad_view(padB4, 0, 32, 32), _pair_view(padB4, 0, 32, 32)
            C4r, C4 = _pad_view(padC4, 0, 32, 32), _pair_view(padC4, 0, 32, 32)
            A5r, A5 = _pad_view(padA5, 0, 16, 16), _pair_view(padA5, 0, 16, 16)
            B5r, B5 = _pad_view(padB5, 0, 16, 16), _pair_view(padB5, 0, 16, 16)
            C5r, C5 = _pad_view(padC5, 0, 16, 16), _pair_view(padC5, 0, 16, 16)

            def wl(s, l, src, dst, H, W, tag, fine_tf=False, rb=None):
                return _WL(nc, wwts_pool, upool, spool, psum_pool, vw_d,
                           sbias, s, l, src, dst, H, W, tag, fine_tf, rb)

            # PE warm-up: HAM gates the PE to 1.2GHz until it has seen
            # ~3.4us of sustained activity. Burn dummy accumulates into
            # one PSUM bank during the startup-DMA window so the first
            # real matmuls run at 2.4GHz. No consumer needed (Tile has
            # no DCE); the bank frees at the last write, long before the
            # first band needs its 8th bank.
            warm = res_pool.tile([P, P], BF16, name="warm")
            wps = psum_pool.tile([P, 16, 32], F32, tag="ps", name="warmps")
            nc.vector.memset(warm[:], 0.0)
            for i in range(48):
                nc.tensor.matmul(wps[:, 0:4, :], warm[:], warm[:],
                                 start=(i == 0), stop=(i == 47))

            # p3 scratch rings: B3/C3 up front; A3 (first read ~60us in) is
            # zeroed after layer 0 so the startup vector queue stays clear
            _zero_ring(nc, _pad_view(padB3, 0, 64, 64), 64, 64)
            _zero_ring(nc, _pad_view(padC3, 0, 64, 64), 64, 64)

            # ---- startup DMAs ----
            nc.scalar.dma_start(
                sbias[:],
                sb_d[:].rearrange("s l a p o -> p (s l a o)")
                       .rearrange("p (s l a o) -> p s l a o",
                                  s=2, l=NL, a=NCH))
            nc.scalar.dma_start(pwc[:], pwc_d[:])
            nc.scalar.dma_start(pwb[:], pwb_d[:])
            nc.scalar.dma_start(pbc[0:20], pbc_d[:])
            nc.scalar.dma_start(pbc[64:84], pbc_d[:])
            nc.scalar.dma_start(pbb[32:37], pbb_d[:])
            nc.scalar.dma_start(pbb[96:101], pbb_d[:])

            # p3 pass: cls l0 F->B, box l0 F->C, cls B->A->B->A (tower A),
            # box C->B->C->B (tower B)
            F3 = feat3[:]
            p3 = [wl(0, 0, F3, B3, 64, 64, "a00", fine_tf=True, rb=8),
                  wl(1, 0, F3, C3, 64, 64, "a10", rb=8),
                  wl(0, 1, B3, A3, 64, 64, "a01"),
                  wl(0, 2, A3, B3, 64, 64, "a02"),
                  wl(0, 3, B3, A3, 64, 64, "a03"),
                  wl(1, 1, C3, B3, 64, 64, "a11"),
                  wl(1, 2, B3, C3, 64, 64, "a12"),
                  wl(1, 3, C3, B3, 64, 64, "a13")]

            # coarse startup DMAs: descriptor ISSUE costs ~650ns each on the
            # queue engine, so fewer/bigger transfers win at startup
            def _feat_rows(r0, r1, eng):
                eng.dma_start(feat3[:, :, r0:r1], x_d[0][:, :, r0:r1])

            # All big DMAs share the sync queue in ascending-need order.
            _feat_rows(0, 18, nc.sync)
            p3[0].weights_pt(0)
            p3[0].weights_pt(1)
            _feat_rows(18, 34, nc.sync)
            _feat_rows(34, 50, nc.sync)
            p3[0].weights()
            _feat_rows(50, 66, nc.sync)
            nc.sync.dma_start(A4r[:, :], x_d[1][:])
            nc.sync.dma_start(A5r[:, :], x_d[2][:])

            # layer 0 of both stems shares one input transform (same src)
            _emit_pair(p3[0], p3[1], nxt=p3[2])
            _zero_ring(nc, _pad_view(padA3, 0, 64, 64), 64, 64)
            _zero_ring(nc, _pad_view(padB4, 0, 32, 32), 32, 32)
            _zero_ring(nc, _pad_view(padC4, 0, 32, 32), 32, 32)
            _zero_ring(nc, _pad_view(padB5, 0, 16, 16), 16, 16)
            _zero_ring(nc, _pad_view(padC5, 0, 16, 16), 16, 16)
            _emit_chain(p3[2:])

            _preds4(nc, psum_pool, stage_pool, pwc, pwb, pbc, pbb,
                    A3r, B3r, out_d, 64, 64, 8, 0, "a")

            # p4/p5 pass: cls l0 A->B, box l0 A->C, then in-place
            # (towers: cls=B, box=C); p4/p5 layers interleaved for slack
            p4 = [wl(0, 0, A4, B4, 32, 32, "b00"),
                  wl(1, 0, A4, C4, 32, 32, "b10"),
                  wl(0, 1, B4, B4, 32, 32, "b01"),
                  wl(1, 1, C4, C4, 32, 32, "b11"),
                  wl(0, 2, B4, B4, 32, 32, "b02"),
                  wl(1, 2, C4, C4, 32, 32, "b12"),
                  wl(0, 3, B4, B4, 32, 32, "b03"),
                  wl(1, 3, C4, C4, 32, 32, "b13")]
            p5 = [wl(0, 0, A5, B5, 16, 16, "c00"),
                  wl(1, 0, A5, C5, 16, 16, "c10"),
                  wl(0, 1, B5, B5, 16, 16, "c01"),
                  wl(1, 1, C5, C5, 16, 16, "c11"),
                  wl(0, 2, B5, B5, 16, 16, "c02"),
                  wl(1, 2, C5, C5, 16, 16, "c12"),
                  wl(0, 3, B5, B5, 16, 16, "c03"),
                  wl(1, 3, C5, C5, 16, 16, "c13")]

            _emit_pair(p4[0], p4[1])
            _emit_pair(p5[0], p5[1])
            for i in range(2, 8):
                for L in (p4[i], p5[i]):
                    L.weights()
                    L.tf(0)
                    L.mm(0)
                    L.outtf(0)

            _preds4(nc, psum_pool, stage_pool, pwc, pwb, pbc, pbb,
                    B4r, C4r, out_d, 32, 32, 16, 4096, "b")
            _preds4(nc, psum_pool, stage_pool, pwc, pwb, pbc, pbb,
                    B5r, C5r, out_d, 16, 16, 16, 5120, "c")

    nc.compile()
    return nc


def _pack_wino_w(wcls, wbox):
    # [s, l, co, ci, ky, kx] -> wino V [s, l, pt, cip, cic, coc, ky, cop]
    w = np.stack([wcls, wbox]).astype(np.float32)   # [2, NL, 256, 256, 3, 3]
    V = np.stack([w[..., 0],
                  (w[..., 0] + w[..., 1] + w[..., 2]) * 0.5,
                  (w[..., 0] - w[..., 1] + w[..., 2]) * 0.5,
                  w[..., 2]], axis=-1)              # [2, NL, co, ci, ky, pt]
    V = V.reshape(2, NL, NCH, P, NCH, P, 3, 4)      # [s,l,coc,cop,cic,cip,ky,pt]
    V = V.transpose(0, 1, 7, 5, 4, 2, 6, 3)         # [s,l,pt,cip,cic,coc,ky,cop]
    return np.ascontiguousarray(V).astype(ml_dtypes.bfloat16)


def _pack_pred_w(w):
    # [co, ci, ky, kx] -> [cip, cic, tap, co]
    n = w.shape[0]
    w = np.asarray(w, np.float32).reshape(n, NCH, P, 3, 3)
    w = w.transpose(2, 1, 3, 4, 0)
    return np.ascontiguousarray(w.reshape(P, NCH, 9, n)).astype(
        ml_dtypes.bfloat16)


def kernel(p3, p4, p5, stem_cls_w, stem_cls_b, stem_box_w, stem_box_b,
           pred_cls_w, pred_cls_b, pred_box_w, pred_box_b,
           pred_ctr_w, pred_ctr_b):
    if 'nc' not in _cached:
        _cached['nc'] = _build()
    nc = _cached['nc']

    B = p3.shape[0]
    vw = _pack_wino_w(np.asarray(stem_cls_w), np.asarray(stem_box_w))
    sb = np.ascontiguousarray(
        np.stack([stem_cls_b, stem_box_b]).reshape(2, NL, NCH, P, 1),
        dtype=np.float32)
    pwc = _pack_pred_w(np.asarray(pred_cls_w))
    pwb = _pack_pred_w(np.concatenate([pred_box_w, pred_ctr_w], axis=0))
    pbc = np.asarray(pred_cls_b, np.float32).reshape(20, 1)
    pbb = np.concatenate([pred_box_b, pred_ctr_b]).astype(np.float32).reshape(5, 1)

    shared = {"vw": vw, "sb": sb, "pwc": pwc, "pwb": pwb,
              "pbc": pbc, "pbb": pbb}
    xs = [np.asarray(p3, np.float32), np.asarray(p4, np.float32),
          np.asarray(p5, np.float32)]
    in_maps = []
    for b in range(B):
        m = dict(shared)
        for i, x in enumerate(xs):
            xp = np.pad(x[b].reshape(NCH, P, x.shape[2], x.shape[3]),
                        ((0, 0), (0, 0), (1, 1), (1, 1)))
            xp = np.ascontiguousarray(xp.transpose(1, 0, 2, 3))
            m[f"x{i}"] = xp.astype(ml_dtypes.bfloat16)
        in_maps.append(m)

    res = run_bass_kernel_spmd(nc, in_maps, core_ids=list(range(B)),
                               **_run_opts)
    _last['res'] = res
    out = np.stack([r["out"].T for r in res.results])
    return np.ascontiguousarray(out, dtype=np.float32)


# revision 48
# speedup vs baseline: 1.0088x; 1.0088x over previous
"""FCOS head (nn_FCOS_73787538145418) Trainium2 Bass kernel.

Sharding: data-parallel, one image per NeuronCore (B=8 across 8 cores),
weights replicated. Each core runs the identical SPMD NEFF over its image.

Stem convs (two 4-layer 3x3 conv 256->256 + ReLU stems per level) run as
1D Winograd F(2,3) along x: per output-column pair, 4 Winograd points
m0..m3 accumulate in separate PSUM banks over (2 ci chunks x 3 ky taps)
K=128 matmuls on bf16 transformed inputs/weights (1.5x fewer PE rows than
direct conv), then y0=relu(m0+m1+m2+b), y1=relu(m1-m2-m3+b) recombine on
the vector/gpsimd/scalar engines while the PE streams the next band.
Towers are stored bf16 (halves transform read cost + DVE write cost).
Prediction convs (cls 20ch; box+ctr 5ch) run direct bf16, 4-way packed
on the PE via tile_position col-groups: each tile's rows split in half;
cls halves accumulate concurrently in col-strips 0/2 (PSUM partitions
0:20 / 64:84) and box+ctr halves in strips 1/3 (32:37 / 96:101). The
four strips stream concurrently (~4 cols/cycle), the per-strip halves
cover different output pixels so no cross-partition combine is needed
(~4x pred throughput vs serial taps). Both stems' layer 0 share one
input transform (same source features).

Hard-won scheduling facts: tensor_tensor may read at most ONE PSUM
operand; gpsimd cannot touch PSUM at all and runs TT at ~half DVE rate;
engines are lane-aligned (no cross-partition compute), so each packed
pred stream keeps all its taps in ONE col group; DMA writes to one tile
are WAW-ordered across queues (ship them on one queue in need-order);
weight pool tiles must be DMA'd on the sync queue; per-matmul issue
overhead is ~6-26ns, LDWEIGHTS (FWL) hides under the matmul stream.
"""
import sys

if '/opt/trn_rl_repo' not in sys.path:
    sys.path.insert(0, '/opt/trn_rl_repo')

import numpy as np
import ml_dtypes

import concourse.mybir as mybir
from concourse import bacc
import concourse.tile as tile
from concourse.bass_utils import run_bass_kernel_spmd

P = 128
NCH = 2                 # 256 channels = 2 chunks of 128
C = 256
NL = 4                  # stem depth
NPIX_TOTAL = 5376
F32 = mybir.dt.float32
BF16 = mybir.dt.bfloat16
ADD = mybir.AluOpType.add
SUB = mybir.AluOpType.subtract
RELU = mybir.ActivationFunctionType.Relu

_cached = {}
_run_opts = {}   # extra kwargs for run_bass_kernel_spmd (test harness: trace)
_last = {}       # last BassKernelResults (test harness reads exec_time_ns)


def _pad_view(flat_tile, off, H, W):
    n = NCH * (H + 2) * (W + 2)
    return flat_tile[:, off:off + n].rearrange(
        "p (c h w) -> p c h w", c=NCH, h=H + 2, w=W + 2)


def _pair_view(flat_tile, off, H, W):
    n = NCH * (H + 2) * (W + 2)
    return flat_tile[:, off:off + n].rearrange(
        "p (c h x two) -> p c h x two",
        c=NCH, h=H + 2, x=(W + 2) // 2, two=2)


def _zero_ring(nc, v, H, W):
    for c in range(NCH):
        nc.vector.memset(v[:, c, 0, :], 0.0)
        nc.vector.memset(v[:, c, H + 1, :], 0.0)
        nc.vector.memset(v[:, c, 1:H + 1, 0], 0.0)
        nc.vector.memset(v[:, c, 1:H + 1, W + 1], 0.0)


class _WL:
    """One Winograd stem conv layer (3x3 same, 256->256, + bias + ReLU)."""

    def __init__(self, nc, wpool, upool, spool, psum, vw_d, sbias,
                 s, l, src_pv, dst_pv, H, W, tag, fine_tf=False, rb=None):
        self.nc = nc
        self.wpool, self.upool, self.spool, self.psum = \
            wpool, upool, spool, psum
        self.vw_d, self.sbias = vw_d, sbias
        self.s, self.l = s, l
        self.src, self.dst = src_pv, dst_pv
        self.H, self.W, self.tag = H, W, tag
        self.fine_tf = fine_tf
        self.RB = rb or (16 if H == 64 else H)   # band rows
        self.NB = H // self.RB              # bands
        self.TX = W // 2                    # tiles per row
        self.RR = self.RB + 2               # U rows per band
        self.wts = {}
        self.us = {}
        self.ms = {}

    def weights_pt(self, pt, eng=None):
        if pt in self.wts:
            return
        nc = self.nc
        eng = eng or nc.sync
        wt = self.wpool.tile([P, NCH, NCH, 3, P], BF16, tag="ww",
                             name=f"ww_{self.tag}_{pt}")
        eng.dma_start(wt[:], self.vw_d[self.s, self.l, pt])
        self.wts[pt] = wt

    def weights(self, eng=None):
        for pt in range(4):
            self.weights_pt(pt, eng)

    def tf(self, q):
        """Input transform for band q: U[c, r, pt, tx] (bf16).

        One op per Winograd point covering both ci chunks; pt0-2 on the
        vector engine, pt3 on gpsimd (gpsimd TT runs at ~half DVE rate
        and also carries the tP/tM output-transform combines)."""
        if q in self.us:
            return
        nc = self.nc
        TX = self.TX
        u = self.upool.tile([P, NCH, self.RR, 4, TX], BF16, tag="u",
                            name=f"u_{self.tag}_{q}")
        r0 = self.RB * q
        rows = slice(r0, r0 + self.RR)
        if self.fine_tf:
            # startup-critical: smaller ops, spread over both engines
            for c in range(NCH):
                e0 = self.src[:, c, rows, 0:TX, 0]
                e1 = self.src[:, c, rows, 1:TX + 1, 0]
                o0 = self.src[:, c, rows, 0:TX, 1]
                o1 = self.src[:, c, rows, 1:TX + 1, 1]
                nc.vector.tensor_tensor(u[:, c, :, 0], e0, e1, SUB)
                nc.vector.tensor_tensor(u[:, c, :, 1], o0, e1, ADD)
                nc.vector.tensor_tensor(u[:, c, :, 2], e1, o0, SUB)
                nc.gpsimd.tensor_tensor(u[:, c, :, 3], o0, o1, SUB)
        else:
            e0 = self.src[:, :, rows, 0:TX, 0]
            e1 = self.src[:, :, rows, 1:TX + 1, 0]
            o0 = self.src[:, :, rows, 0:TX, 1]
            o1 = self.src[:, :, rows, 1:TX + 1, 1]
            nc.vector.tensor_tensor(u[:, :, :, 0], e0, e1, SUB)
            nc.gpsimd.tensor_tensor(u[:, :, :, 1], o0, e1, ADD)
            nc.gpsimd.tensor_tensor(u[:, :, :, 2], e1, o0, SUB)
            nc.gpsimd.tensor_tensor(u[:, :, :, 3], o0, o1, SUB)
        self.us[q] = u

    def mm(self, b):
        nc = self.nc
        u = self.us[b]
        self.ms[b] = {}
        sz = self.RB * self.TX
        for o in range(NCH):
            if sz <= 256:
                # half-band mode: pack two winograd points per 2KB bank
                # (pool PSUM allocation is bank-granular) so a paired
                # layer's matmuls can overlap in the other 4 banks.
                # Pair (m1,m2) / (m0,m3): freed together by the out-tf.
                slot = {1: 0, 2: 1, 0: 2, 3: 3}
                banks = [self.psum.tile([P, 512], F32, tag="ps",
                                        name=f"mb_{self.tag}_{b}_{o}_{i}")
                         for i in range(2)]
                self.ms[b][o] = [
                    banks[slot[pt] // 2]
                    [:, (slot[pt] % 2) * sz:(slot[pt] % 2) * sz + sz]
                    .rearrange("p (r x) -> p r x", r=self.RB, x=self.TX)
                    for pt in range(4)]
            else:
                self.ms[b][o] = [
                    self.psum.tile([P, self.RB, self.TX], F32, tag="ps",
                                   name=f"m_{self.tag}_{b}_{o}_{pt}")
                    for pt in range(4)]
        # pt-outer order: each point's weight tile is only needed 2*NCH
        # matmuls later than with o-outer, giving the weight-DMA prefetch
        # ~2.6us more slack (kills the startup/layer-boundary PE gaps)
        for pt in range(4):
            wt = self.wts[pt]
            for o in range(NCH):
                k = 0
                for c in range(NCH):
                    for ky in range(3):
                        nc.tensor.matmul(self.ms[b][o][pt][:],
                                         wt[:, c, o, ky],
                                         u[:, c, ky:ky + self.RB, pt],
                                         start=(k == 0), stop=(k == 5))
                        k += 1

    def outtf(self, b):
        """y0 = relu(m0+m1+m2+b) -> odd cols; y1 = relu(m1-m2-m3+b) -> even."""
        nc = self.nc
        RB, TX = self.RB, self.TX
        rows = slice(1 + RB * b, 1 + RB * (b + 1))
        for o in range(NCH):
            m0, m1, m2, m3 = self.ms[b][o]
            t = f"{self.tag}_{b}_{o}"
            c2 = self.spool.tile([P, RB, TX], F32, tag="scf", name=f"c2_{t}")
            tP = self.spool.tile([P, RB, TX], F32, tag="scf", name=f"tp_{t}")
            tM = self.spool.tile([P, RB, TX], F32, tag="scf", name=f"tm_{t}")
            r0 = self.spool.tile([P, RB, TX], F32, tag="scf", name=f"r0_{t}")
            r1 = self.spool.tile([P, RB, TX], F32, tag="scf", name=f"r1_{t}")
            bias = self.sbias[:, self.s, self.l, o]
            # short PSUM-freeing chain on ONE fast engine: a long
            # cross-engine chain here stalls the next layer's matmuls on
            # PSUM banks and lets HAM re-throttle the PE (measured -120us)
            nc.scalar.copy(c2[:], m2[:])
            nc.vector.tensor_tensor(tP[:], m1[:], c2[:], ADD)
            nc.vector.tensor_tensor(tM[:], m1[:], c2[:], SUB)
            nc.vector.tensor_tensor(r0[:], m0[:], tP[:], ADD)
            nc.vector.tensor_tensor(r1[:], m3[:], tM[:], SUB)   # m3 - tM
            nc.scalar.activation(self.dst[:, o, rows, 0:TX, 1], r0[:],
                                 RELU, bias=bias)
            nc.scalar.activation(self.dst[:, o, rows, 1:TX + 1, 0], r1[:],
                                 RELU, bias=bias, scale=-1.0)
        del self.ms[b]


def _emit_chain(layers, post_hooks=None):
    """Emit a list of _WL layers sequentially with next-layer tf hoisting."""
    post_hooks = post_hooks or {}
    n = len(layers)
    for i, L in enumerate(layers):
        nxt = layers[i + 1] if i + 1 < n else None
        L.weights()
        L.tf(0)
        if L.NB > 1:
            L.tf(1)
        for b in range(L.NB):
            L.mm(b)
            if b + 2 < L.NB:
                L.tf(b + 2)
            if b == L.NB - 1 and nxt is not None:
                nxt.weights()
                nxt.tf(0)
            L.outtf(b)
            if b == L.NB - 1 and nxt is not None and nxt.NB > 1:
                nxt.tf(1)
        if i in post_hooks:
            post_hooks[i]()


def _emit_pair(Lc, Lb, nxt=None):
    """Emit two stem layers that read the SAME source (both stems'
    layer 0), sharing one input transform: Lb reuses Lc's U tiles and
    the two layers' matmuls/out-transforms interleave band-by-band."""
    Lb.us = Lc.us     # share the U dict by reference
    Lc.weights()
    Lb.weights()
    Lc.tf(0)
    if Lc.NB > 1:
        Lc.tf(1)
    for b in range(Lc.NB):
        Lc.mm(b)
        if b + 2 < Lc.NB:
            Lc.tf(b + 2)
        Lc.outtf(b)
        Lb.mm(b)
        if b == Lc.NB - 1 and nxt is not None:
            nxt.weights()
        Lb.outtf(b)
    if nxt is not None:
        nxt.tf(0)
        if nxt.NB > 1:
            nxt.tf(1)


def _preds4(nc, psum_pool, stage_pool, pwc, pwb, pbc, pbb,
            tcls, tbox, out_d, H, W, R, pix_base, tag):
    """Packed prediction convs, 4 concurrent PE col-strips per tile:
    each tile's rows split in half; cls halves accumulate in strips 0/2
    (PSUM partitions 0:20 / 64:84), box+ctr halves in strips 1/3
    (32:37 / 96:101). Different output pixels per strip, so no
    cross-partition combine is needed — ACT adds bias per strip and the
    DMAs reassemble rows in DRAM."""
    R2 = R // 2
    n_tiles = H // R
    IDENT = mybir.ActivationFunctionType.Identity
    for it in range(n_tiles):
        rr = it * R
        ps = psum_pool.tile([P, R, W], F32, tag="ps", name=f"pf_{tag}_{it}")
        for k in range(18):
            c, t = k // 9, k % 9
            ky, kx = t // 3, t % 3
            rc0 = tcls[:, c, rr + ky:rr + ky + R2, kx:kx + W]
            rc1 = tcls[:, c, rr + R2 + ky:rr + R2 + ky + R2, kx:kx + W]
            rb0 = tbox[:, c, rr + ky:rr + ky + R2, kx:kx + W]
            rb1 = tbox[:, c, rr + R2 + ky:rr + R2 + ky + R2, kx:kx + W]
            st_, sp_ = (k == 0), (k == 17)
            nc.tensor.matmul(ps[0:20, 0:R2], pwc[:, c, t], rc0,
                             start=st_, stop=sp_, tile_position=(0, 0))
            nc.tensor.matmul(ps[32:37, 0:R2], pwb[:, c, t], rb0,
                             start=st_, stop=sp_, tile_position=(0, 32))
            nc.tensor.matmul(ps[64:84, R2:R], pwc[:, c, t], rc1,
                             start=st_, stop=sp_, tile_position=(0, 64))
            nc.tensor.matmul(ps[96:101, R2:R], pwb[:, c, t], rb1,
                             start=st_, stop=sp_, tile_position=(0, 96))
        st = stage_pool.tile([P, R * W], F32, tag="st", name=f"st_{tag}_{it}")
        n2 = R2 * W
        c0 = pix_base + rr * W
        for half, (pc, pb) in enumerate([(0, 32), (64, 96)]):
            vc = ps[pc:pc + 20, half * R2:half * R2 + R2].rearrange(
                "p r w -> p (r w)")
            vb = ps[pb:pb + 5, half * R2:half * R2 + R2].rearrange(
                "p r w -> p (r w)")
            sc = st[pc:pc + 20, half * n2:half * n2 + n2]
            sb = st[pb:pb + 5, half * n2:half * n2 + n2]
            nc.scalar.activation(sc, vc, IDENT, bias=pbc[pc:pc + 20])
            nc.scalar.activation(sb, vb, IDENT, bias=pbb[pb:pb + 5])
            nc.sync.dma_start(
                out_d[0:20, c0 + half * n2:c0 + half * n2 + n2], sc)
            nc.sync.dma_start(
                out_d[20:25, c0 + half * n2:c0 + half * n2 + n2], sb)


def _build():
    nc = bacc.Bacc("TRN2", target_bir_lowering=False, debug=False,
                   num_devices=8)

    # all features ship bf16 (they only feed winograd transforms / preds)
    x_d = [nc.dram_tensor("x0", (P, NCH, 66, 66), BF16,
                          kind="ExternalInput"),
           nc.dram_tensor("x1", (P, NCH, 34, 34), BF16,
                          kind="ExternalInput"),
           nc.dram_tensor("x2", (P, NCH, 18, 18), BF16,
                          kind="ExternalInput")]
    vw_d = nc.dram_tensor("vw", (2, NL, 4, P, NCH, NCH, 3, P), BF16,
                          kind="ExternalInput")
    sb_d = nc.dram_tensor("sb", (2, NL, NCH, P, 1), F32, kind="ExternalInput")
    pwc_d = nc.dram_tensor("pwc", (P, NCH, 9, 20), BF16, kind="ExternalInput")
    pwb_d = nc.dram_tensor("pwb", (P, NCH, 9, 5), BF16, kind="ExternalInput")
    pbc_d = nc.dram_tensor("pbc", (20, 1), F32, kind="ExternalInput")
    pbb_d = nc.dram_tensor("pbb", (5, 1), F32, kind="ExternalInput")
    out_d = nc.dram_tensor("out", (25, NPIX_TOTAL), F32, kind="ExternalOutput")

    N3 = NCH * 66 * 66            # 8712: p3 padded elems/partition
    N4 = NCH * 34 * 34            # 2312
    N5 = NCH * 18 * 18            # 648

    with tile.TileContext(nc) as tc:
        with (
            tc.tile_pool(name="resident", bufs=1) as res_pool,
            tc.tile_pool(name="wwts", bufs=12) as wwts_pool,
            tc.tile_pool(name="upool", bufs=3) as upool,
            tc.tile_pool(name="scratch", bufs=8) as spool,
            tc.tile_pool(name="psum", bufs=8, space="PSUM") as psum_pool,
            tc.tile_pool(name="stage", bufs=4) as stage_pool,
        ):
            # p3 rotation buffers (bf16): A3 holds the cls tower, B3/C3
            # rotate for the box chain; feat3 holds the p3 features.
            padA3 = res_pool.tile([P, N3], BF16, name="padA3")
            padB3 = res_pool.tile([P, N3], BF16, name="padB3")
            padC3 = res_pool.tile([P, N3], BF16, name="padC3")
            feat3 = res_pool.tile([P, NCH, 66, 33, 2], BF16, name="feat3")
            # p4/p5 get their own (non-aliased) buffers so the scheduler
            # can overlap p4/p5 stems with the p3 preds.
            padA4 = res_pool.tile([P, N4], BF16, name="padA4")
            padB4 = res_pool.tile([P, N4], BF16, name="padB4")
            padC4 = res_pool.tile([P, N4], BF16, name="padC4")
            padA5 = res_pool.tile([P, N5], BF16, name="padA5")
            padB5 = res_pool.tile([P, N5], BF16, name="padB5")
            padC5 = res_pool.tile([P, N5], BF16, name="padC5")

            sbias = res_pool.tile([P, 2, NL, NCH, 1], F32, name="sbias")
            pwc = res_pool.tile([P, NCH, 9, 20], BF16, name="pwc")
            pwb = res_pool.tile([P, NCH, 9, 5], BF16, name="pwb")
            pbc = res_pool.tile([96, 1], F32, name="pbc")
            pbb = res_pool.tile([P, 1], F32, name="pbb")

            A3r, A3 = _pad_view(padA3, 0, 64, 64), _pair_view(padA3, 0, 64, 64)
            B3r, B3 = _pad_view(padB3, 0, 64, 64), _pair_view(padB3, 0, 64, 64)
            C3 = _pair_view(padC3, 0, 64, 64)
            A4r, A4 = _pad_view(padA4, 0, 32, 32), _pair_view(padA4, 0, 32, 32)
            B4r, B4 = _pad_view(padB4, 0, 32, 32), _pair_view(padB4, 0, 32, 32)
            C4r, C4 = _pad_view(padC4, 0, 32, 32), _pair_view(padC4, 0, 32, 32)
            A5r, A5 = _pad_view(padA5, 0, 16, 16), _pair_view(padA5, 0, 16, 16)
            B5r, B5 = _pad_view(padB5, 0, 16, 16), _pair_view(padB5, 0, 16, 16)
            C5r, C5 = _pad_view(padC5, 0, 16, 16), _pair_view(padC5, 0, 16, 16)

            def wl(s, l, src, dst, H, W, tag, fine_tf=False, rb=None):
                return _WL(nc, wwts_pool, upool, spool, psum_pool, vw_d,
                           sbias, s, l, src, dst, H, W, tag, fine_tf, rb)

            # PE warm-up: HAM gates the PE to 1.2GHz until it has seen
            # ~3.4us of sustained activity. Burn dummy accumulates into
            # one PSUM bank during the startup-DMA window so the first
            # real matmuls run at 2.4GHz. No consumer needed (Tile has
            # no DCE); the bank frees at the last write, long before the
            # first band needs its 8th bank.
            warm = res_pool.tile([P, P], BF16, name="warm")
            wps = psum_pool.tile([P, 16, 32], F32, tag="ps", name="warmps")
            nc.vector.memset(warm[:], 0.0)
            for i in range(48):
                nc.tensor.matmul(wps[:, 0:4, :], warm[:], warm[:],
                                 start=(i == 0), stop=(i == 47))

            # p3 scratch rings: B3/C3 up front; A3 (first read ~60us in) is
            # zeroed after layer 0 so the startup vector queue stays clear
            _zero_ring(nc, _pad_view(padB3, 0, 64, 64), 64, 64)
            _zero_ring(nc, _pad_view(padC3, 0, 64, 64), 64, 64)

            # ---- startup DMAs ----
            nc.scalar.dma_start(
                sbias[:],
                sb_d[:].rearrange("s l a p o -> p (s l a o)")
                       .rearrange("p (s l a o) -> p s l a o",
                                  s=2, l=NL, a=NCH))
            nc.scalar.dma_start(pwc[:], pwc_d[:])
            nc.scalar.dma_start(pwb[:], pwb_d[:])
            nc.scalar.dma_start(pbc[0:20], pbc_d[:])
            nc.scalar.dma_start(pbc[64:84], pbc_d[:])
            nc.scalar.dma_start(pbb[32:37], pbb_d[:])
            nc.scalar.dma_start(pbb[96:101], pbb_d[:])

            # p3 pass: cls l0 F->B, box l0 F->C, cls B->A->B->A (tower A),
            # box C->B->C->B (tower B)
            F3 = feat3[:]
            p3 = [wl(0, 0, F3, B3, 64, 64, "a00", fine_tf=True, rb=8),
                  wl(1, 0, F3, C3, 64, 64, "a10", rb=8),
                  wl(0, 1, B3, A3, 64, 64, "a01"),
                  wl(0, 2, A3, B3, 64, 64, "a02"),
                  wl(0, 3, B3, A3, 64, 64, "a03"),
                  wl(1, 1, C3, B3, 64, 64, "a11"),
                  wl(1, 2, B3, C3, 64, 64, "a12"),
                  wl(1, 3, C3, B3, 64, 64, "a13")]

            # coarse startup DMAs: descriptor ISSUE costs ~650ns each on the
            # queue engine, so fewer/bigger transfers win at startup
            def _feat_rows(r0, r1, eng):
                eng.dma_start(feat3[:, :, r0:r1], x_d[0][:, :, r0:r1])

            # All big DMAs share the sync queue in ascending-need order.
            # With half-band layer-0 pairs, feature chunks 2-4 aren't
            # consumed until ~30us+ in, so BOTH stems' layer-0 weights
            # (needed from ~13us/~19us) go right after the first chunk.
            _feat_rows(0, 18, nc.sync)
            p3[0].weights()
            p3[1].weights()
            _feat_rows(18, 34, nc.sync)
            _feat_rows(34, 50, nc.sync)
            _feat_rows(50, 66, nc.sync)
            nc.sync.dma_start(A4r[:, :], x_d[1][:])
            nc.sync.dma_start(A5r[:, :], x_d[2][:])

            # layer 0 of both stems shares one input transform (same src)
            _emit_pair(p3[0], p3[1], nxt=p3[2])
            _zero_ring(nc, _pad_view(padA3, 0, 64, 64), 64, 64)
            _zero_ring(nc, _pad_view(padB4, 0, 32, 32), 32, 32)
            _zero_ring(nc, _pad_view(padC4, 0, 32, 32), 32, 32)
            _zero_ring(nc, _pad_view(padB5, 0, 16, 16), 16, 16)
            _zero_ring(nc, _pad_view(padC5, 0, 16, 16), 16, 16)
            _emit_chain(p3[2:])

            _preds4(nc, psum_pool, stage_pool, pwc, pwb, pbc, pbb,
                    A3r, B3r, out_d, 64, 64, 8, 0, "a")

            # p4/p5 pass: cls l0 A->B, box l0 A->C, then in-place
            # (towers: cls=B, box=C); p4/p5 layers interleaved for slack
            p4 = [wl(0, 0, A4, B4, 32, 32, "b00"),
                  wl(1, 0, A4, C4, 32, 32, "b10"),
                  wl(0, 1, B4, B4, 32, 32, "b01"),
                  wl(1, 1, C4, C4, 32, 32, "b11"),
                  wl(0, 2, B4, B4, 32, 32, "b02"),
                  wl(1, 2, C4, C4, 32, 32, "b12"),
                  wl(0, 3, B4, B4, 32, 32, "b03"),
                  wl(1, 3, C4, C4, 32, 32, "b13")]
            p5 = [wl(0, 0, A5, B5, 16, 16, "c00"),
                  wl(1, 0, A5, C5, 16, 16, "c10"),
                  wl(0, 1, B5, B5, 16, 16, "c01"),
                  wl(1, 1, C5, C5, 16, 16, "c11"),
                  wl(0, 2, B5, B5, 16, 16, "c02"),
                  wl(1, 2, C5, C5, 16, 16, "c12"),
                  wl(0, 3, B5, B5, 16, 16, "c03"),
                  wl(1, 3, C5, C5, 16, 16, "c13")]

            _emit_pair(p4[0], p4[1])
            _emit_pair(p5[0], p5[1])
            for i in range(2, 8):
                for L in (p4[i], p5[i]):
                    L.weights()
                    L.tf(0)
                    L.mm(0)
                    L.outtf(0)

            _preds4(nc, psum_pool, stage_pool, pwc, pwb, pbc, pbb,
                    B4r, C4r, out_d, 32, 32, 16, 4096, "b")
            _preds4(nc, psum_pool, stage_pool, pwc, pwb, pbc, pbb,
                    B5r, C5r, out_d, 16, 16, 16, 5120, "c")

    nc.compile()
    return nc


def _pack_wino_w(wcls, wbox):
    # [s, l, co, ci, ky, kx] -> wino V [s, l, pt, cip, cic, coc, ky, cop]
    w = np.stack([wcls, wbox]).astype(np.float32)   # [2, NL, 256, 256, 3, 3]
    V = np.stack([w[..., 0],
                  (w[..., 0] + w[..., 1] + w[..., 2]) * 0.5,
                  (w[..., 0] - w[..., 1] + w[..., 2]) * 0.5,
                  w[..., 2]], axis=-1)              # [2, NL, co, ci, ky, pt]
    V = V.reshape(2, NL, NCH, P, NCH, P, 3, 4)      # [s,l,coc,cop,cic,cip,ky,pt]
    V = V.transpose(0, 1, 7, 5, 4, 2, 6, 3)         # [s,l,pt,cip,cic,coc,ky,cop]
    return np.ascontiguousarray(V).astype(ml_dtypes.bfloat16)


def _pack_pred_w(w):
    # [co, ci, ky, kx] -> [cip, cic, tap, co]
    n = w.shape[0]
    w = np.asarray(w, np.float32).reshape(n, NCH, P, 3, 3)
    w = w.transpose(2, 1, 3, 4, 0)
    return np.ascontiguousarray(w.reshape(P, NCH, 9, n)).astype(
        ml_dtypes.bfloat16)


def kernel(p3, p4, p5, stem_cls_w, stem_cls_b, stem_box_w, stem_box_b,
           pred_cls_w, pred_cls_b, pred_box_w, pred_box_b,
           pred_ctr_w, pred_ctr_b):
    if 'nc' not in _cached:
        _cached['nc'] = _build()
    nc = _cached['nc']

    B = p3.shape[0]
    vw = _pack_wino_w(np.asarray(stem_cls_w), np.asarray(stem_box_w))
    sb = np.ascontiguousarray(
        np.stack([stem_cls_b, stem_box_b]).reshape(2, NL, NCH, P, 1),
        dtype=np.float32)
    pwc = _pack_pred_w(np.asarray(pred_cls_w))
    pwb = _pack_pred_w(np.concatenate([pred_box_w, pred_ctr_w], axis=0))
    pbc = np.asarray(pred_cls_b, np.float32).reshape(20, 1)
    pbb = np.concatenate([pred_box_b, pred_ctr_b]).astype(np.float32).reshape(5, 1)

    shared = {"vw": vw, "sb": sb, "pwc": pwc, "pwb": pwb,
              "pbc": pbc, "pbb": pbb}
    xs = [np.asarray(p3, np.float32), np.asarray(p4, np.float32),
          np.asarray(p5, np.float32)]
    in_maps = []
    for b in range(B):
        m = dict(shared)
        for i, x in enumerate(xs):
            xp = np.pad(x[b].reshape(NCH, P, x.shape[2], x.shape[3]),
                        ((0, 0), (0, 0), (1, 1), (1, 1)))
            xp = np.ascontiguousarray(xp.transpose(1, 0, 2, 3))
            m[f"x{i}"] = xp.astype(ml_dtypes.bfloat16)
        in_maps.append(m)

    res = run_bass_kernel_spmd(nc, in_maps, core_ids=list(range(B)),
                               **_run_opts)
    _last['res'] = res
    out = np.stack([r["out"].T for r in res.results])
    return np.ascontiguousarray(out, dtype=np.float32)
